# revision 13
# baseline (speedup 1.0000x reference)
"""ForwardWarpDWeight (bilinear splat forward warp) on 8 trn2 NeuronCores.

Pure data parallel per the sharding hint: batch element b runs on core b.

Hybrid single-NEFF-call pipeline, fused in ONE jitted shard_map:
  1. Bass kernel (custom call, ~320us on-device): all elementwise work on
     DVE/ACT — flow clip, depth-weight exp, bilinear corner weights +
     validity, per-corner contribution payloads and flat cell indices.
  2. XLA scatter-add (segment_sum) of the 4*H*W contribution rows — the
     only part issued through the device compiler, since this container's
     Q7 vector-indirect descriptor generator mispairs idx<->payload for
     multi-index CCE scatters (verified by byte-level forensics), ruling
     out a raw Bass scatter.
  3. Normalize by the splatted depth weight + mask, reshape to [3,H,W].
Data stays on-device between stages (no host bounce).
"""
import os
import sys

for _p in ("/opt/trn_rl_repo", "/root/.axon_site/_ro/trn_rl_repo"):
    if os.path.isdir(_p) and _p not in sys.path:
        sys.path.insert(0, _p)

import numpy as np

B, C, H, W = 8, 3, 384, 1280
HW = H * W
NCH = 5
WSUB = 320
NSUB_W = W // WSUB
NROWG = H // 128

_JITTED = None


def _build_bass():
    sys.path.insert(0, "/root/problem/work")
    import concourse.bass as bass
    import concourse.mybir as mybir
    from concourse.tile import TileContext

    F32 = mybir.dt.float32
    I32 = mybir.dt.int32
    tt = mybir.AluOpType

    nc = bass.Bass()
    x_in = nc.declare_dram_parameter("x", [C, H, W], F32, isOutput=False)
    fl_in = nc.declare_dram_parameter("flow", [2, H, W], F32, isOutput=False)
    dp_in = nc.declare_dram_parameter("depth", [1, H, W], F32, isOutput=False)
    con_t = nc.declare_dram_parameter("con", [NROWG, NSUB_W, 4, 128, WSUB * NCH],
                                      F32, isOutput=True)
    idx_t = nc.declare_dram_parameter("idx", [NROWG, NSUB_W, 4, 128, WSUB],
                                      I32, isOutput=True)

    def _subtile(pool, g, sw):
        r0 = g * 128
        c0 = sw * WSUB
        WS = WSUB

        def wtile(tag, shape=None, dtype=F32):
            return pool.tile(shape or [128, WS], dtype, tag=tag, name=tag)

        fx = wtile("fx"); fy = wtile("fy"); dep = wtile("dep")
        nc.sync.dma_start(out=fx[:], in_=fl_in[0, r0:r0 + 128, c0:c0 + WS])
        nc.sync.dma_start(out=fy[:], in_=fl_in[1, r0:r0 + 128, c0:c0 + WS])
        nc.sync.dma_start(out=dep[:], in_=dp_in[0, r0:r0 + 128, c0:c0 + WS])
        xc = []
        for c in range(C):
            t = wtile(f"xc{c}")
            nc.sync.dma_start(out=t[:], in_=x_in[c, r0:r0 + 128, c0:c0 + WS])
            xc.append(t)

        wcol_i = wtile("wcol_i", dtype=I32)
        nc.gpsimd.iota(wcol_i[:], pattern=[[1, WS]], base=c0, channel_multiplier=0)
        h_i = wtile("h_i", [128, 1], I32)
        nc.gpsimd.iota(h_i[:], pattern=[[1, 1]], base=r0, channel_multiplier=1)
        wcol_f = wtile("wcol_f")
        nc.vector.tensor_copy(out=wcol_f[:], in_=wcol_i[:])
        h_f = wtile("h_f", [128, 1])
        nc.vector.tensor_copy(out=h_f[:], in_=h_i[:])

        nc.vector.tensor_scalar(out=fx[:], in0=fx[:], scalar1=-2560.0,
                                scalar2=2560.0, op0=tt.max, op1=tt.min)
        nc.vector.tensor_scalar(out=fy[:], in0=fy[:], scalar1=-2560.0,
                                scalar2=2560.0, op0=tt.max, op1=tt.min)
        xs = wtile("xs"); ys = wtile("ys")
        nc.vector.tensor_tensor(out=xs[:], in0=fx[:], in1=wcol_f[:], op=tt.add)
        nc.vector.tensor_scalar(out=ys[:], in0=fy[:], scalar1=h_f[:, :1],
                                scalar2=None, op0=tt.add)

        def floor_of(src, tag):
            fi = wtile(tag + "_i", dtype=I32)
            nc.vector.tensor_copy(out=fi[:], in_=src[:])
            ff = wtile(tag + "_f")
            nc.vector.tensor_copy(out=ff[:], in_=fi[:])
            gt = wtile(tag + "_gt")
            nc.vector.tensor_tensor(out=gt[:], in0=ff[:], in1=src[:], op=tt.is_gt)
            nc.vector.tensor_tensor(out=ff[:], in0=ff[:], in1=gt[:], op=tt.subtract)
            return ff

        x0f = floor_of(xs, "x0")
        y0f = floor_of(ys, "y0")
        fxr = wtile("fxr"); fyr = wtile("fyr")
        nc.vector.tensor_tensor(out=fxr[:], in0=xs[:], in1=x0f[:], op=tt.subtract)
        nc.vector.tensor_tensor(out=fyr[:], in0=ys[:], in1=y0f[:], op=tt.subtract)
        wx0 = wtile("wx0"); wy0 = wtile("wy0")
        nc.vector.tensor_scalar(out=wx0[:], in0=fxr[:], scalar1=-1.0, scalar2=1.0,
                                op0=tt.mult, op1=tt.add)
        nc.vector.tensor_scalar(out=wy0[:], in0=fyr[:], scalar1=-1.0, scalar2=1.0,
                                op0=tt.mult, op1=tt.add)

        v = wtile("v"); vt = wtile("vt")
        nc.vector.tensor_scalar(out=v[:], in0=xs[:], scalar1=0.0, scalar2=None,
                                op0=tt.is_ge)
        nc.vector.tensor_scalar(out=vt[:], in0=xs[:], scalar1=1279.0, scalar2=None,
                                op0=tt.is_lt)
        nc.vector.tensor_tensor(out=v[:], in0=v[:], in1=vt[:], op=tt.mult)
        nc.vector.tensor_scalar(out=vt[:], in0=ys[:], scalar1=0.0, scalar2=None,
                                op0=tt.is_ge)
        nc.vector.tensor_tensor(out=v[:], in0=v[:], in1=vt[:], op=tt.mult)
        nc.vector.tensor_scalar(out=vt[:], in0=ys[:], scalar1=383.0, scalar2=None,
                                op0=tt.is_lt)
        nc.vector.tensor_tensor(out=v[:], in0=v[:], in1=vt[:], op=tt.mult)

        nc.vector.tensor_scalar(out=dep[:], in0=dep[:], scalar1=0.001, scalar2=80.0,
                                op0=tt.max, op1=tt.min)
        nc.vector.tensor_scalar(out=dep[:], in0=dep[:], scalar1=-0.2, scalar2=8.0,
                                op0=tt.mult, op1=tt.add)
        dw = wtile("dw")
        nc.scalar.activation(out=dw[:], in_=dep[:],
                             func=mybir.ActivationFunctionType.Exp)

        wy0v = wtile("wy0v", [128, WS, 1])
        wy1v = wtile("wy1v", [128, WS, 1])
        nc.vector.tensor_tensor(out=wy0v[:, :, 0], in0=wy0[:], in1=v[:], op=tt.mult)
        nc.vector.tensor_tensor(out=wy1v[:, :, 0], in0=fyr[:], in1=v[:], op=tt.mult)
        wx03 = wtile("wx03", [128, WS, 1])
        fxr3 = wtile("fxr3", [128, WS, 1])
        nc.vector.tensor_copy(out=wx03[:, :, 0], in_=wx0[:])
        nc.vector.tensor_copy(out=fxr3[:, :, 0], in_=fxr[:])

        s_t = wtile("s_t", [128, WS, NCH])
        nc.vector.tensor_copy(out=s_t[:, :, 0], in_=dw[:])
        for c in range(C):
            nc.vector.tensor_tensor(out=s_t[:, :, 1 + c], in0=xc[c][:], in1=dw[:],
                                    op=tt.mult)
        nc.vector.tensor_scalar(out=s_t[:, :, NCH - 1], in0=dw[:], scalar1=0.0,
                                scalar2=1.0, op0=tt.mult, op1=tt.add)

        sy0 = wtile("sy0", [128, WS, NCH])
        sy1 = wtile("sy1", [128, WS, NCH])
        nc.vector.tensor_tensor(out=sy0[:], in0=s_t[:],
                                in1=wy0v[:].to_broadcast([128, WS, NCH]), op=tt.mult)
        nc.vector.tensor_tensor(out=sy1[:], in0=s_t[:],
                                in1=wy1v[:].to_broadcast([128, WS, NCH]), op=tt.mult)
        for ci, (nm, syt, wxt) in enumerate((("nw", sy0, wx03), ("ne", sy0, fxr3),
                                             ("sw", sy1, wx03), ("se", sy1, fxr3))):
            pt = wtile("pay_" + nm, [128, WS, NCH])
            nc.vector.tensor_tensor(out=pt[:], in0=syt[:],
                                    in1=wxt[:].to_broadcast([128, WS, NCH]),
                                    op=tt.mult)
            nc.sync.dma_start(out=con_t[g, sw, ci],
                              in_=pt[:].rearrange("p w c -> p (w c)"))

        idxf = wtile("idxf")
        nc.vector.tensor_scalar(out=idxf[:], in0=y0f[:], scalar1=1280.0,
                                scalar2=None, op0=tt.mult)
        nc.vector.tensor_tensor(out=idxf[:], in0=idxf[:], in1=x0f[:], op=tt.add)
        nc.vector.tensor_tensor(out=idxf[:], in0=idxf[:], in1=v[:], op=tt.mult)
        vw = wtile("vw")
        nc.vector.tensor_scalar(out=vw[:], in0=v[:], scalar1=1280.0, scalar2=None,
                                op0=tt.mult)
        tmpf = wtile("tmpf")
        for ci, nm in enumerate(("nw", "ne", "sw", "se")):
            it = wtile("idx_" + nm, dtype=I32)
            if nm == "nw":
                nc.vector.tensor_copy(out=it[:], in_=idxf[:])
            elif nm == "ne":
                nc.vector.tensor_tensor(out=tmpf[:], in0=idxf[:], in1=v[:], op=tt.add)
                nc.vector.tensor_copy(out=it[:], in_=tmpf[:])
            elif nm == "sw":
                nc.vector.tensor_tensor(out=tmpf[:], in0=idxf[:], in1=vw[:], op=tt.add)
                nc.vector.tensor_copy(out=it[:], in_=tmpf[:])
            else:
                nc.vector.tensor_tensor(out=tmpf[:], in0=idxf[:], in1=vw[:], op=tt.add)
                nc.vector.tensor_tensor(out=tmpf[:], in0=tmpf[:], in1=v[:], op=tt.add)
                nc.vector.tensor_copy(out=it[:], in_=tmpf[:])
            nc.sync.dma_start(out=idx_t[g, sw, ci], in_=it[:])

    with TileContext(nc) as tc:
        with tc.tile_pool(name="work", bufs=2) as pool:
            for g in range(NROWG):
                for sw in range(NSUB_W):
                    _subtile(pool, g, sw)

    # walrus here allows only one sync wait per instruction
    counter = 0
    for fn in nc.m.functions:
        for bb in fn.blocks:
            out = []
            for inst in bb.instructions:
                si = inst.sync_info
                if si is not None and si.on_wait and len(si.on_wait) > 1:
                    waits = list(si.on_wait)
                    for sw_ in waits[:-1]:
                        counter += 1
                        w = mybir.InstEventSemaphore(
                            name=f"WSPL-{counter}", ins=[], outs=[],
                            sync_info=mybir.SyncInfo(on_wait=[sw_], on_update=[]))
                        w.engine = inst.engine
                        out.append(w)
                    si.on_wait = [waits[-1]]
                out.append(inst)
            bb.instructions[:] = out
    return nc


def _build():
    global _JITTED
    if _JITTED is not None:
        return _JITTED
    import jax
    import jax.numpy as jnp
    from jax.sharding import Mesh, PartitionSpec
    from jax.experimental.shard_map import shard_map
    from concourse.bass2jax import (_bass_exec_p, install_neuronx_cc_hook,
                                    partition_id_tensor)
    import concourse.mybir as mybir

    install_neuronx_cc_hook()
    nc = _build_bass()

    in_names = []
    out_names = []
    out_avals = []
    zero_shapes = []
    partition_name = nc.partition_id_tensor.name if nc.partition_id_tensor else None
    for alloc in nc.m.functions[0].allocations:
        if not isinstance(alloc, mybir.MemoryLocationSet):
            continue
        name = alloc.memorylocations[0].name
        if alloc.kind == "ExternalInput":
            if name != partition_name:
                in_names.append(name)
        elif alloc.kind == "ExternalOutput":
            shape = tuple(alloc.tensor_shape)
            dtype = mybir.dt.np(alloc.dtype)
            out_names.append(name)
            out_avals.append(jax.core.ShapedArray(shape, dtype))
            zero_shapes.append((shape, dtype))
    all_names = list(in_names) + list(out_names)
    if partition_name is not None:
        all_names.append(partition_name)

    name_order = {"x": 0, "flow": 1, "depth": 2}
    assert sorted(in_names, key=lambda n: name_order[n]) == ["x", "flow", "depth"]

    def prep_shard(x, flow, depth, zcon, zidx):
        # shards arrive exactly in BIR-declared per-core shapes (no reshape!)
        args = {"x": x, "flow": flow, "depth": depth}
        operands = [args[n] for n in in_names]
        operands += [{"con": zcon, "idx": zidx}[n] for n in out_names]
        if partition_name is not None:
            operands.append(partition_id_tensor())
        outs = _bass_exec_p.bind(
            *operands,
            out_avals=tuple(out_avals),
            in_names=tuple(all_names),
            out_names=tuple(out_names),
            lowering_input_output_aliases=(),
            sim_require_finite=True,
            sim_require_nnan=True,
            nc=nc,
        )
        res = dict(zip(out_names, outs))
        return res["con"], res["idx"]

    def scat_shard(con, idx):
        seg = jax.ops.segment_sum(con.reshape(-1, NCH), idx.reshape(-1),
                                  num_segments=HW)   # [HW, 5]
        dwf = seg[:, 0]
        msk = seg[:, 4]
        inv = jnp.where(msk < 0.5, 0.0, 1.0) / jnp.maximum(dwf, 1e-7)
        return (seg[:, 1:4] * inv[:, None]).T.reshape(1, C, H, W)

    devices = jax.devices()[:B]
    mesh = Mesh(np.asarray(devices), ("b",))
    P = PartitionSpec
    fn1 = jax.jit(
        shard_map(prep_shard, mesh=mesh, in_specs=(P("b"),) * 5,
                  out_specs=(P("b"), P("b")), check_rep=False),
        donate_argnums=(3, 4), keep_unused=True,
    )
    fn2 = jax.jit(
        shard_map(scat_shard, mesh=mesh, in_specs=(P("b"), P("b")),
                  out_specs=P("b"), check_rep=False)
    )

    def fn(x, flow, depth):
        # concat per-core inputs on axis 0 so each device shard matches the
        # BIR-declared per-core shape with no reshape (hook requirement)
        xc = x.reshape(B * C, H, W)
        fc = flow.reshape(B * 2, H, W)
        dc = depth.reshape(B * 1, H, W)
        zcon = np.zeros((B * NROWG, NSUB_W, 4, 128, WSUB * NCH), np.float32)
        zidx = np.zeros((B * NROWG, NSUB_W, 4, 128, WSUB), np.int32)
        con, idx = fn1(xc, fc, dc, zcon, zidx)
        return fn2(con, idx)

    _JITTED = fn
    return fn


def kernel(x, flow, depth):
    import jax

    fn = _build()
    x = np.ascontiguousarray(x, dtype=np.float32)
    flow = np.ascontiguousarray(flow, dtype=np.float32)
    depth = np.ascontiguousarray(depth, dtype=np.float32)
    out = np.asarray(jax.block_until_ready(fn(x, flow, depth)))
    return out.astype(np.float32)


# revision 14
# speedup vs baseline: 2.2073x; 2.2073x over previous
"""ForwardWarpDWeight (bilinear splat forward warp) on 8 trn2 NeuronCores.

Pure data parallel per the sharding hint: batch element b runs on core b.

Hybrid single-NEFF-call pipeline, fused in ONE jitted shard_map:
  1. Bass kernel (custom call, ~320us on-device): all elementwise work on
     DVE/ACT — flow clip, depth-weight exp, bilinear corner weights +
     validity, per-corner contribution payloads and flat cell indices.
  2. XLA scatter-add (segment_sum) of the 4*H*W contribution rows — the
     only part issued through the device compiler, since this container's
     Q7 vector-indirect descriptor generator mispairs idx<->payload for
     multi-index CCE scatters (verified by byte-level forensics), ruling
     out a raw Bass scatter.
  3. Normalize by the splatted depth weight + mask, reshape to [3,H,W].
Data stays on-device between stages (no host bounce).
"""
import os
import sys

for _p in ("/opt/trn_rl_repo", "/root/.axon_site/_ro/trn_rl_repo"):
    if os.path.isdir(_p) and _p not in sys.path:
        sys.path.insert(0, _p)

import numpy as np

B, C, H, W = 8, 3, 384, 1280
HW = H * W
NCH = 5
WSUB = 320
NSUB_W = W // WSUB
NROWG = H // 128

_JITTED = None


def _build_bass():
    sys.path.insert(0, "/root/problem/work")
    import concourse.bass as bass
    import concourse.mybir as mybir
    from concourse.tile import TileContext

    F32 = mybir.dt.float32
    I32 = mybir.dt.int32
    tt = mybir.AluOpType

    nc = bass.Bass()
    x_in = nc.declare_dram_parameter("x", [C, H, W], F32, isOutput=False)
    fl_in = nc.declare_dram_parameter("flow", [2, H, W], F32, isOutput=False)
    dp_in = nc.declare_dram_parameter("depth", [1, H, W], F32, isOutput=False)
    con_t = nc.declare_dram_parameter("con", [NROWG, NSUB_W, 4, 128, WSUB * NCH],
                                      F32, isOutput=True)
    idx_t = nc.declare_dram_parameter("idx", [NROWG, NSUB_W, 4, 128, WSUB],
                                      I32, isOutput=True)

    def _subtile(pool, g, sw):
        r0 = g * 128
        c0 = sw * WSUB
        WS = WSUB

        def wtile(tag, shape=None, dtype=F32):
            return pool.tile(shape or [128, WS], dtype, tag=tag, name=tag)

        fx = wtile("fx"); fy = wtile("fy"); dep = wtile("dep")
        nc.sync.dma_start(out=fx[:], in_=fl_in[0, r0:r0 + 128, c0:c0 + WS])
        nc.sync.dma_start(out=fy[:], in_=fl_in[1, r0:r0 + 128, c0:c0 + WS])
        nc.sync.dma_start(out=dep[:], in_=dp_in[0, r0:r0 + 128, c0:c0 + WS])
        xc = []
        for c in range(C):
            t = wtile(f"xc{c}")
            nc.sync.dma_start(out=t[:], in_=x_in[c, r0:r0 + 128, c0:c0 + WS])
            xc.append(t)

        wcol_i = wtile("wcol_i", dtype=I32)
        nc.gpsimd.iota(wcol_i[:], pattern=[[1, WS]], base=c0, channel_multiplier=0)
        h_i = wtile("h_i", [128, 1], I32)
        nc.gpsimd.iota(h_i[:], pattern=[[1, 1]], base=r0, channel_multiplier=1)
        wcol_f = wtile("wcol_f")
        nc.vector.tensor_copy(out=wcol_f[:], in_=wcol_i[:])
        h_f = wtile("h_f", [128, 1])
        nc.vector.tensor_copy(out=h_f[:], in_=h_i[:])

        nc.vector.tensor_scalar(out=fx[:], in0=fx[:], scalar1=-2560.0,
                                scalar2=2560.0, op0=tt.max, op1=tt.min)
        nc.vector.tensor_scalar(out=fy[:], in0=fy[:], scalar1=-2560.0,
                                scalar2=2560.0, op0=tt.max, op1=tt.min)
        xs = wtile("xs"); ys = wtile("ys")
        nc.vector.tensor_tensor(out=xs[:], in0=fx[:], in1=wcol_f[:], op=tt.add)
        nc.vector.tensor_scalar(out=ys[:], in0=fy[:], scalar1=h_f[:, :1],
                                scalar2=None, op0=tt.add)

        def floor_of(src, tag):
            fi = wtile(tag + "_i", dtype=I32)
            nc.vector.tensor_copy(out=fi[:], in_=src[:])
            ff = wtile(tag + "_f")
            nc.vector.tensor_copy(out=ff[:], in_=fi[:])
            gt = wtile(tag + "_gt")
            nc.vector.tensor_tensor(out=gt[:], in0=ff[:], in1=src[:], op=tt.is_gt)
            nc.vector.tensor_tensor(out=ff[:], in0=ff[:], in1=gt[:], op=tt.subtract)
            return ff

        x0f = floor_of(xs, "x0")
        y0f = floor_of(ys, "y0")
        fxr = wtile("fxr"); fyr = wtile("fyr")
        nc.vector.tensor_tensor(out=fxr[:], in0=xs[:], in1=x0f[:], op=tt.subtract)
        nc.vector.tensor_tensor(out=fyr[:], in0=ys[:], in1=y0f[:], op=tt.subtract)
        wx0 = wtile("wx0"); wy0 = wtile("wy0")
        nc.vector.tensor_scalar(out=wx0[:], in0=fxr[:], scalar1=-1.0, scalar2=1.0,
                                op0=tt.mult, op1=tt.add)
        nc.vector.tensor_scalar(out=wy0[:], in0=fyr[:], scalar1=-1.0, scalar2=1.0,
                                op0=tt.mult, op1=tt.add)

        v = wtile("v"); vt = wtile("vt")
        nc.vector.tensor_scalar(out=v[:], in0=xs[:], scalar1=0.0, scalar2=None,
                                op0=tt.is_ge)
        nc.vector.tensor_scalar(out=vt[:], in0=xs[:], scalar1=1279.0, scalar2=None,
                                op0=tt.is_lt)
        nc.vector.tensor_tensor(out=v[:], in0=v[:], in1=vt[:], op=tt.mult)
        nc.vector.tensor_scalar(out=vt[:], in0=ys[:], scalar1=0.0, scalar2=None,
                                op0=tt.is_ge)
        nc.vector.tensor_tensor(out=v[:], in0=v[:], in1=vt[:], op=tt.mult)
        nc.vector.tensor_scalar(out=vt[:], in0=ys[:], scalar1=383.0, scalar2=None,
                                op0=tt.is_lt)
        nc.vector.tensor_tensor(out=v[:], in0=v[:], in1=vt[:], op=tt.mult)

        nc.vector.tensor_scalar(out=dep[:], in0=dep[:], scalar1=0.001, scalar2=80.0,
                                op0=tt.max, op1=tt.min)
        nc.vector.tensor_scalar(out=dep[:], in0=dep[:], scalar1=-0.2, scalar2=8.0,
                                op0=tt.mult, op1=tt.add)
        dw = wtile("dw")
        nc.scalar.activation(out=dw[:], in_=dep[:],
                             func=mybir.ActivationFunctionType.Exp)

        wy0v = wtile("wy0v", [128, WS, 1])
        wy1v = wtile("wy1v", [128, WS, 1])
        nc.vector.tensor_tensor(out=wy0v[:, :, 0], in0=wy0[:], in1=v[:], op=tt.mult)
        nc.vector.tensor_tensor(out=wy1v[:, :, 0], in0=fyr[:], in1=v[:], op=tt.mult)
        wx03 = wtile("wx03", [128, WS, 1])
        fxr3 = wtile("fxr3", [128, WS, 1])
        nc.vector.tensor_copy(out=wx03[:, :, 0], in_=wx0[:])
        nc.vector.tensor_copy(out=fxr3[:, :, 0], in_=fxr[:])

        s_t = wtile("s_t", [128, WS, NCH])
        nc.vector.tensor_copy(out=s_t[:, :, 0], in_=dw[:])
        for c in range(C):
            nc.vector.tensor_tensor(out=s_t[:, :, 1 + c], in0=xc[c][:], in1=dw[:],
                                    op=tt.mult)
        nc.vector.tensor_scalar(out=s_t[:, :, NCH - 1], in0=dw[:], scalar1=0.0,
                                scalar2=1.0, op0=tt.mult, op1=tt.add)

        sy0 = wtile("sy0", [128, WS, NCH])
        sy1 = wtile("sy1", [128, WS, NCH])
        nc.vector.tensor_tensor(out=sy0[:], in0=s_t[:],
                                in1=wy0v[:].to_broadcast([128, WS, NCH]), op=tt.mult)
        nc.vector.tensor_tensor(out=sy1[:], in0=s_t[:],
                                in1=wy1v[:].to_broadcast([128, WS, NCH]), op=tt.mult)
        for ci, (nm, syt, wxt) in enumerate((("nw", sy0, wx03), ("ne", sy0, fxr3),
                                             ("sw", sy1, wx03), ("se", sy1, fxr3))):
            pt = wtile("pay_" + nm, [128, WS, NCH])
            nc.vector.tensor_tensor(out=pt[:], in0=syt[:],
                                    in1=wxt[:].to_broadcast([128, WS, NCH]),
                                    op=tt.mult)
            nc.sync.dma_start(out=con_t[g, sw, ci],
                              in_=pt[:].rearrange("p w c -> p (w c)"))

        idxf = wtile("idxf")
        nc.vector.tensor_scalar(out=idxf[:], in0=y0f[:], scalar1=1280.0,
                                scalar2=None, op0=tt.mult)
        nc.vector.tensor_tensor(out=idxf[:], in0=idxf[:], in1=x0f[:], op=tt.add)
        nc.vector.tensor_tensor(out=idxf[:], in0=idxf[:], in1=v[:], op=tt.mult)
        vw = wtile("vw")
        nc.vector.tensor_scalar(out=vw[:], in0=v[:], scalar1=1280.0, scalar2=None,
                                op0=tt.mult)
        tmpf = wtile("tmpf")
        for ci, nm in enumerate(("nw", "ne", "sw", "se")):
            it = wtile("idx_" + nm, dtype=I32)
            if nm == "nw":
                nc.vector.tensor_copy(out=it[:], in_=idxf[:])
            elif nm == "ne":
                nc.vector.tensor_tensor(out=tmpf[:], in0=idxf[:], in1=v[:], op=tt.add)
                nc.vector.tensor_copy(out=it[:], in_=tmpf[:])
            elif nm == "sw":
                nc.vector.tensor_tensor(out=tmpf[:], in0=idxf[:], in1=vw[:], op=tt.add)
                nc.vector.tensor_copy(out=it[:], in_=tmpf[:])
            else:
                nc.vector.tensor_tensor(out=tmpf[:], in0=idxf[:], in1=vw[:], op=tt.add)
                nc.vector.tensor_tensor(out=tmpf[:], in0=tmpf[:], in1=v[:], op=tt.add)
                nc.vector.tensor_copy(out=it[:], in_=tmpf[:])
            nc.sync.dma_start(out=idx_t[g, sw, ci], in_=it[:])

    with TileContext(nc) as tc:
        with tc.tile_pool(name="work", bufs=2) as pool:
            for g in range(NROWG):
                for sw in range(NSUB_W):
                    _subtile(pool, g, sw)

    # walrus here allows only one sync wait per instruction
    counter = 0
    for fn in nc.m.functions:
        for bb in fn.blocks:
            out = []
            for inst in bb.instructions:
                si = inst.sync_info
                if si is not None and si.on_wait and len(si.on_wait) > 1:
                    waits = list(si.on_wait)
                    for sw_ in waits[:-1]:
                        counter += 1
                        w = mybir.InstEventSemaphore(
                            name=f"WSPL-{counter}", ins=[], outs=[],
                            sync_info=mybir.SyncInfo(on_wait=[sw_], on_update=[]))
                        w.engine = inst.engine
                        out.append(w)
                    si.on_wait = [waits[-1]]
                out.append(inst)
            bb.instructions[:] = out
    return nc


def _build():
    global _JITTED
    if _JITTED is not None:
        return _JITTED
    import jax
    import jax.numpy as jnp
    from jax.sharding import Mesh, PartitionSpec
    from jax.experimental.shard_map import shard_map
    from concourse.bass2jax import (_bass_exec_p, install_neuronx_cc_hook,
                                    partition_id_tensor)
    import concourse.mybir as mybir

    install_neuronx_cc_hook()
    nc = _build_bass()

    in_names = []
    out_names = []
    out_avals = []
    zero_shapes = []
    partition_name = nc.partition_id_tensor.name if nc.partition_id_tensor else None
    for alloc in nc.m.functions[0].allocations:
        if not isinstance(alloc, mybir.MemoryLocationSet):
            continue
        name = alloc.memorylocations[0].name
        if alloc.kind == "ExternalInput":
            if name != partition_name:
                in_names.append(name)
        elif alloc.kind == "ExternalOutput":
            shape = tuple(alloc.tensor_shape)
            dtype = mybir.dt.np(alloc.dtype)
            out_names.append(name)
            out_avals.append(jax.core.ShapedArray(shape, dtype))
            zero_shapes.append((shape, dtype))
    all_names = list(in_names) + list(out_names)
    if partition_name is not None:
        all_names.append(partition_name)

    name_order = {"x": 0, "flow": 1, "depth": 2}
    assert sorted(in_names, key=lambda n: name_order[n]) == ["x", "flow", "depth"]

    def prep_shard(x, flow, depth, zcon, zidx):
        # shards arrive exactly in BIR-declared per-core shapes (no reshape!)
        args = {"x": x, "flow": flow, "depth": depth}
        operands = [args[n] for n in in_names]
        operands += [{"con": zcon, "idx": zidx}[n] for n in out_names]
        if partition_name is not None:
            operands.append(partition_id_tensor())
        outs = _bass_exec_p.bind(
            *operands,
            out_avals=tuple(out_avals),
            in_names=tuple(all_names),
            out_names=tuple(out_names),
            lowering_input_output_aliases=(),
            sim_require_finite=True,
            sim_require_nnan=True,
            nc=nc,
        )
        res = dict(zip(out_names, outs))
        return res["con"], res["idx"]

    def scat_shard(con, idx):
        seg = jax.ops.segment_sum(con.reshape(-1, NCH), idx.reshape(-1),
                                  num_segments=HW)   # [HW, 5]
        dwf = seg[:, 0]
        msk = seg[:, 4]
        inv = jnp.where(msk < 0.5, 0.0, 1.0) / jnp.maximum(dwf, 1e-7)
        return (seg[:, 1:4] * inv[:, None]).T.reshape(1, C, H, W)

    devices = jax.devices()[:B]
    mesh = Mesh(np.asarray(devices), ("b",))
    P = PartitionSpec
    fn1 = jax.jit(
        shard_map(prep_shard, mesh=mesh, in_specs=(P("b"),) * 5,
                  out_specs=(P("b"), P("b")), check_rep=False),
        keep_unused=True,
    )
    fn2 = jax.jit(
        shard_map(scat_shard, mesh=mesh, in_specs=(P("b"), P("b")),
                  out_specs=P("b"), check_rep=False)
    )

    def fn(x, flow, depth):
        # concat per-core inputs on axis 0 so each device shard matches the
        # BIR-declared per-core shape with no reshape (hook requirement)
        xc = x.reshape(B * C, H, W)
        fc = flow.reshape(B * 2, H, W)
        dc = depth.reshape(B * 1, H, W)
        if not hasattr(fn, "_z"):
            fn._z = (jnp.zeros((B * NROWG, NSUB_W, 4, 128, WSUB * NCH), jnp.float32),
                     jnp.zeros((B * NROWG, NSUB_W, 4, 128, WSUB), jnp.int32))
        con, idx = fn1(xc, fc, dc, *fn._z)
        return fn2(con, idx)

    _JITTED = fn
    return fn


def kernel(x, flow, depth):
    import jax

    fn = _build()
    x = np.ascontiguousarray(x, dtype=np.float32)
    flow = np.ascontiguousarray(flow, dtype=np.float32)
    depth = np.ascontiguousarray(depth, dtype=np.float32)
    out = np.asarray(jax.block_until_ready(fn(x, flow, depth)))
    return out.astype(np.float32)


# revision 15
# speedup vs baseline: 7.9870x; 3.6184x over previous
"""ForwardWarpDWeight (bilinear splat forward warp) on 8 trn2 NeuronCores.

Pure data parallel per the sharding hint: batch element b runs on core b;
each splat is independent per batch element so there is no cross-device
scatter traffic. The warp (clip, depth-weight exp, bilinear corner
weights + validity, fused 5-channel scatter-add splat, normalize) runs
on-device, sharded over the 8 cores with shard_map.

Note: a hand-written Bass pipeline (DVE elementwise + CCE-add indirect-DMA
scatter into engine-private HBM accumulators) was prototyped first, but this
container's Q7 vector-indirect descriptor generator mispairs idx<->payload
for multi-index-per-partition scatters (payloads stream from the first
index instead of honoring per-descriptor addresses), which corrupts any
per-pixel scatter. The splat is therefore issued through the device
compiler's scatter-add path, which is correct on this hardware.
"""
import os
import sys

import numpy as np

B, C, H, W = 8, 3, 384, 1280
REF_SCALE = 5.0

_JITTED = None


def _build():
    global _JITTED
    if _JITTED is not None:
        return _JITTED
    import jax
    import jax.numpy as jnp
    from jax.sharding import Mesh, PartitionSpec
    from jax.experimental.shard_map import shard_map

    def _forward_warp(im, flow):
        # im: [b,Ch,H,W], flow: [b,H,W,2]
        b, Ch, Hh, Ww = im.shape
        xs = flow[..., 0] + jnp.arange(Ww, dtype=flow.dtype)
        ys = flow[..., 1] + jnp.arange(Hh, dtype=flow.dtype)[:, None]
        x0 = jnp.floor(xs); y0 = jnp.floor(ys)
        x1 = x0 + 1.0;      y1 = y0 + 1.0
        valid = (x0 >= 0) & (x1 <= Ww - 1) & (y0 >= 0) & (y1 <= Hh - 1)
        x0i = x0.astype(jnp.int32); y0i = y0.astype(jnp.int32)
        x1i = x1.astype(jnp.int32); y1i = y1.astype(jnp.int32)
        w_nw = (x1 - xs) * (y1 - ys)
        w_ne = (xs - x0) * (y1 - ys)
        w_sw = (x1 - xs) * (ys - y0)
        w_se = (xs - x0) * (ys - y0)

        # fuse the 4 corner splats into ONE scatter-add call
        idxs = []
        contribs = []
        for w_c, yi, xi in ((w_nw, y0i, x0i), (w_ne, y0i, x1i),
                            (w_sw, y1i, x0i), (w_se, y1i, x1i)):
            idx = jnp.where(valid, yi * Ww + xi, 0)          # [b,H,W]
            wv = jnp.where(valid, w_c, jnp.zeros_like(w_c))  # [b,H,W]
            idxs.append(idx.reshape(b, -1))
            contribs.append((im * wv[:, None]).reshape(b, Ch, -1))
        idx_all = jnp.concatenate(idxs, axis=1)                       # [b, 4HW]
        con_all = jnp.concatenate(contribs, axis=2).transpose(0, 2, 1)  # [b,4HW,Ch]

        def splat_one(con_b, idx_b):
            return jax.ops.segment_sum(con_b, idx_b, num_segments=Hh * Ww)

        out = jax.vmap(splat_one)(con_all, idx_all)
        return out.transpose(0, 2, 1).reshape(b, Ch, Hh, Ww)

    def _per_shard(x, flow, depth):
        # local shapes: x [1,3,H,W], flow [1,2,H,W], depth [1,1,H,W]
        flow = jnp.clip(flow, -2.0 * W, 2.0 * W)
        flow = jnp.transpose(flow, (0, 2, 3, 1))
        depth = jnp.clip(depth, 0.001, 80.0)
        depth_weight = jnp.exp(-(depth - 40.0) / REF_SCALE)
        mask = jnp.ones_like(depth)
        stacked = jnp.concatenate([depth_weight, x * depth_weight, mask], axis=1)
        warped = _forward_warp(stacked, flow)
        dw_flowed = warped[:, 0:1]
        xw_flowed = warped[:, 1:1 + C]
        mask_flowed = warped[:, 1 + C:2 + C]
        invalid = mask_flowed < 0.5
        xw_flowed = jnp.where(invalid, jnp.zeros_like(xw_flowed), xw_flowed)
        return xw_flowed / jnp.maximum(dw_flowed, 1e-7)

    devices = jax.devices()[:B]
    mesh = Mesh(np.asarray(devices), ("b",))
    fn = jax.jit(
        shard_map(
            _per_shard,
            mesh=mesh,
            in_specs=(PartitionSpec("b"), PartitionSpec("b"), PartitionSpec("b")),
            out_specs=PartitionSpec("b"),
            check_rep=False,
        )
    )
    _JITTED = fn
    return fn


def kernel(x, flow, depth):
    import jax

    fn = _build()
    x = np.ascontiguousarray(x, dtype=np.float32)
    flow = np.ascontiguousarray(flow, dtype=np.float32)
    depth = np.ascontiguousarray(depth, dtype=np.float32)
    out = fn(x, flow, depth)
    out = np.asarray(jax.block_until_ready(out))
    return out.astype(np.float32)
